# revision 8
# baseline (speedup 1.0000x reference)
"""Trainium2 Bass kernel for nn_Block_49624052138029 (dense transformer block).

Strategy: data parallel over batch x sequence: core i = (batch i//4, query
quarter i%4 of 512 tokens). K/V are computed redundantly on all 4 cores of a
batch group (measured faster than collectives: the gather would sit on the
critical path with nothing to overlap it).

Key-permutation trick: attention is permutation-invariant over keys, so each
core receives xfull ROTATED so its own 512 rows come first. One identical
program then serves all 8 cores: LN/K/V production starts on the rows the
core already owns, Q + the first attention chunks never wait on remote rows.

v2 schedule (this session):
- LN1 runs once over all 2048 rows, own rows first (the old kernel LN'd own
  rows twice); Q is emitted right after the own 4 row-tiles.
- A short warmup stream of junk matmuls pushes HAM to K=8/8 before the first
  transpose (transposes don't count as PE-busy for the clock gate).
- K/V projections run at N=1024 (fp8 DoubleRow, moving [128,2,1024]): half
  the matmul count of N=512, so LDWEIGHTS hides under the matmuls.
- The softmax denominator reciprocal runs batched over groups of 4 heads
  ([4,512] costs the same as [1,512]: DVE reciprocal is ~8 cyc/elem and
  lane-parallel) - the per-head version burned 53us of DVE.
- wo/w1/w2 live in dedicated SBUF tiles whose DMAs are issued before
  attention (the old rotating 2-buf pool serialized the w1/w2 loads behind
  the Wo/W1 consumers: 5us PE gaps + HAM cold windows).
- The final output DMA streams per 128-column strip as the FFN finishes it
  instead of waiting for the whole residual (16us -> ~3us tail).

Precision (validated vs the reference): QKV projections fp8 DoubleRow
(h, Wq/Wk/Wv in fp8e4m3); K^T and packed Q stored fp8; QK on fp8 operands;
exp() evacuation on ScalarE with fp8 out; AV in fp8 DoubleRow with an
all-ones V column so the softmax denominator falls out of the same
accumulation (row 64 of U^T). Post-attention dense layers (Wo/W1/W2) bf16.
"""

import os
import sys

for _p in ("/root/.axon_site", "/root/.axon_site/_ro/trn_rl_repo",
           "/root/.axon_site/_ro/pypackages", "/opt/trn_rl_repo", "/opt/pypackages"):
    if os.path.isdir(_p) and _p not in sys.path:
        sys.path.append(_p)

import numpy as np
import ml_dtypes

import concourse.bass as bass
import concourse.tile as tile
from concourse import bacc, mybir
from concourse.bass_utils import run_bass_kernel_spmd
from concourse.masks import make_identity

F32 = mybir.dt.float32
BF16 = mybir.dt.bfloat16
FP8 = mybir.dt.float8e4
AF = mybir.ActivationFunctionType
ALU = mybir.AluOpType
DR = mybir.MatmulPerfMode.DoubleRow

D = 1024          # model dim
H = 16            # heads
E = 64            # head dim
T = 2048          # tokens per batch
TQ = 512          # tokens owned by this core
P = 128
KO = D // P       # 8 feature chunks
RT_OWN = TQ // P  # 4 row tiles owned
SO = T // P       # 16 key chunks
EPS = 1e-5
SCALE = 1.0 / 32.0  # D ** -0.5

# K/V projection moving-operand width (fp8 DR). 1024 halves the matmul
# count vs 512; flip to 512 if the wide moving operand is rejected.
KV_N = 1024


def _layer_norm_rows(nc, sng, xrow_ap, stats_pool, y_out_ap):
    """Row-major LayerNorm core: y = (x - mean(x)) * rsqrt(var(x) + eps).

    xrow_ap: [128, 1024] f32 SBUF; y_out_ap: [128, 1024] (any dtype) SBUF.
    gamma/beta are NOT applied here (folded into the weights host-side).
    """
    stats = stats_pool.tile([P, 2, 6], F32, tag="bnstats")
    xg = xrow_ap.rearrange("p (g d) -> p g d", g=2)
    for g in range(2):
        nc.vector.bn_stats(out=stats[:, g, :], in_=xg[:, g, :])
    mv = stats_pool.tile([P, 2], F32, tag="bnaggr")
    nc.vector.bn_aggr(out=mv[:], in_=stats[:])
    rstd = stats_pool.tile([P, 1], F32, tag="rstd")
    # rstd = 1 / sqrt(var + eps)
    nc.scalar.activation(out=rstd[:], in_=mv[:, 1:2], func=AF.Sqrt,
                         bias=sng["eps"][:], scale=1.0)
    nc.vector.reciprocal(out=rstd[:], in_=rstd[:])
    nc.vector.tensor_scalar(
        out=y_out_ap, in0=xrow_ap, scalar1=mv[:, 0:1], scalar2=rstd[:],
        op0=ALU.subtract, op1=ALU.mult)


def build_kernel():
    nc = bacc.Bacc(None, target_bir_lowering=False, debug=False, num_devices=8)

    # xfull is PERMUTED per core: own 512 rows first, then the rest.
    xfull = nc.dram_tensor("xfull", [T, D], F32, kind="ExternalInput")
    wq = nc.dram_tensor("wq", [D, D], FP8, kind="ExternalInput")
    wk = nc.dram_tensor("wk", [D, D], FP8, kind="ExternalInput")
    wv = nc.dram_tensor("wv", [D, D], FP8, kind="ExternalInput")
    wo = nc.dram_tensor("wo", [D, D], BF16, kind="ExternalInput")
    w1 = nc.dram_tensor("w1", [D, D], BF16, kind="ExternalInput")
    w2 = nc.dram_tensor("w2", [D, D], BF16, kind="ExternalInput")
    cq = nc.dram_tensor("cq", [D], F32, kind="ExternalInput")
    ck = nc.dram_tensor("ck", [D], F32, kind="ExternalInput")
    bo = nc.dram_tensor("bo", [D], F32, kind="ExternalInput")
    b1 = nc.dram_tensor("b1", [D], F32, kind="ExternalInput")
    b2 = nc.dram_tensor("b2", [D], F32, kind="ExternalInput")
    out = nc.dram_tensor("out", [TQ, D], F32, kind="ExternalOutput")

    # per-feature params as [128, 8] (partition p, chunk o) for feature-major use
    def pm(dram_vec):
        return dram_vec.rearrange("(o p) -> p o", p=P)

    with tile.TileContext(nc) as tc:
        with (
            tc.tile_pool(name="singles", bufs=1) as singles,
            tc.tile_pool(name="persist", bufs=1) as persist,
            tc.tile_pool(name="hrow", bufs=4) as hrow_pool,
            tc.tile_pool(name="stats", bufs=6) as stats_pool,
        ):
            # ---------------- persistent activations + weights ----------------
            kT = persist.tile([P, KO, T], FP8, name="kT")           # 2 MB
            vP = persist.tile([P, SO // 2, 2, H, E + 1], FP8, name="vP")  # 2.08 MB
            qPack = persist.tile([P, KO, 2, TQ], FP8, name="qPack")  # 1 MB
            oT = persist.tile([P, KO, TQ], BF16, name="oT")         # 1 MB
            x1 = persist.tile([P, RT_OWN, D], F32, name="x1")       # 2 MB
            h2T = persist.tile([P, KO, TQ], BF16, name="h2T")       # 1 MB
            fT = persist.tile([P, KO, TQ], BF16, name="fT")         # 1 MB
            hT_full = persist.tile([P, KO, T], FP8, name="hT_full")  # 2 MB

            wqAll = persist.tile([P, KO, D], FP8, name="wqAll")     # 1 MB
            wkAll = persist.tile([P, KO, D], FP8, name="wkAll")     # 1 MB
            wvAll = persist.tile([P, KO, D], FP8, name="wvAll")     # 1 MB
            woAll = persist.tile([P, KO, D], BF16, name="woAll")    # 2 MB
            w1All = persist.tile([P, KO, D], BF16, name="w1All")    # 2 MB
            w2All = persist.tile([P, KO, D], BF16, name="w2All")    # 2 MB

            # x own rows first: LN1 heads every dependency chain
            for rt in range(RT_OWN):
                nc.sync.dma_start(x1[:, rt, :], xfull[rt * P:(rt + 1) * P, :])
            nc.sync.dma_start(wqAll[:], wq.rearrange("(o p) n -> p o n", p=P))
            nc.sync.dma_start(wkAll[:], wk.rearrange("(o p) n -> p o n", p=P))
            nc.sync.dma_start(wvAll[:], wv.rearrange("(o p) n -> p o n", p=P))

            # ---------------- setup ----------------
            sng = {}
            id_bf = singles.tile([P, P], BF16, name="id_bf")
            make_identity(nc, id_bf[:])
            id_f32 = singles.tile([P, P], F32, name="id_f32")
            make_identity(nc, id_f32[:])
            sng["eps"] = singles.tile([P, 1], F32, name="eps")
            nc.vector.memset(sng["eps"][:], EPS)
            ones64 = singles.tile([1, E], BF16, name="ones64")
            nc.vector.memset(ones64[:], 1.0)

            cq_pm = singles.tile([P, KO], F32, name="cq_pm")
            nc.sync.dma_start(cq_pm[:], pm(cq))
            ck_pm = singles.tile([P, KO], F32, name="ck_pm")
            nc.sync.dma_start(ck_pm[:], pm(ck))
            bo_pm = singles.tile([P, KO], F32, name="bo_pm")
            nc.sync.dma_start(bo_pm[:], pm(bo))
            bf1_pm = singles.tile([P, KO], F32, name="bf1_pm")
            nc.sync.dma_start(bf1_pm[:], pm(b1))
            bf2_pm = singles.tile([P, KO], F32, name="bf2_pm")
            nc.sync.dma_start(bf2_pm[:], pm(b2))

            # head-group denominator staging: head h of a group of 4 parks its
            # denominator row at partition 32*(h%4) so one [128,TQ] reciprocal
            # covers 4 heads (DVE reciprocal cost scales with free size only).
            # selK0/selK1 broadcast the dinv rows back over the oT partitions:
            # out[p,q] = sum_k sel[k,p] * dinv[k,q], sel one-hot per 64-row half.
            selK0 = singles.tile([P, P], BF16, name="selK0")
            nc.vector.memset(selK0[:], 0.0)
            nc.vector.memset(selK0[0:1, 0:E], 1.0)
            nc.vector.memset(selK0[32:33, E:P], 1.0)
            selK1 = singles.tile([P, P], BF16, name="selK1")
            nc.vector.memset(selK1[:], 0.0)
            nc.vector.memset(selK1[64:65, 0:E], 1.0)
            nc.vector.memset(selK1[96:97, E:P], 1.0)

            # ---------------- HAM warmup ----------------
            # ~4us of junk matmuls so the PE clock gate is at 8/8 before the
            # first real work (transposes do not count as PE-busy for HAM).
            warm_ctx = tc.tile_pool(name="ps_warm", bufs=2, space="PSUM")
            ps_warm = warm_ctx.__enter__()
            for i in range(48):
                wt = ps_warm.tile([P, E], F32, tag="warm")
                nc.tensor.matmul(wt[:], id_bf[:], id_bf[:, 0:E], start=True, stop=True)
            warm_ctx.__exit__(None, None, None)

            ps_tr_ctx = tc.tile_pool(name="ps_tr", bufs=2, space="PSUM")
            ps_tr = ps_tr_ctx.__enter__()
            ps_w_ctx = tc.tile_pool(name="ps_w", bufs=2, space="PSUM")
            ps_w = ps_w_ctx.__enter__()
            xrow_ctx = tc.tile_pool(name="xrow", bufs=3)
            xrow_pool = xrow_ctx.__enter__()

            # memsets out of the startup critical path (needed from Q/attn on)
            nc.vector.memset(qPack[:], 0.0)
            nc.vector.memset(vP[:, :, :, :, E], 1.0)

            # ---------------- LN1 over all rows -> hT_full ----------------
            def ln_tile_from(x_ap, rt):
                """LayerNorm one 128-row tile (source already in SBUF) and
                scatter its transpose into hT_full columns rt*128..+128."""
                y_row = hrow_pool.tile([P, D], BF16, tag="hrow")
                _layer_norm_rows(nc, sng, x_ap, stats_pool, y_row[:])
                trp = ps_tr.tile([P, KO, P], BF16, tag="tr")
                for ko in range(KO):
                    nc.tensor.transpose(trp[:, ko, :], y_row[:, ko * P:(ko + 1) * P], id_bf[:])
                nc.scalar.copy(out=hT_full[:, :, rt * P:(rt + 1) * P], in_=trp[:])

            def emit_ln_dma(rt):
                x_t = xrow_pool.tile([P, D], F32, tag="xrow")
                nc.sync.dma_start(x_t[:], xfull[rt * P:(rt + 1) * P, :])
                ln_tile_from(x_t[:], rt)

            # own rows (already being DMA'd into x1)
            for rt in range(RT_OWN):
                ln_tile_from(x1[:, rt, :], rt)

            # ---------------- Q over own rows ----------------
            def emit_q(he):
                psq = ps_w.tile([P, TQ], F32, tag="psq", name="psq")
                for jj in range(KO // 2):
                    nc.tensor.matmul(
                        psq[:], wqAll[:, 2 * jj:2 * jj + 2, he * P:(he + 1) * P],
                        hT_full[:, 2 * jj:2 * jj + 2, 0:TQ],
                        start=(jj == 0), stop=(jj == KO // 2 - 1), perf_mode=DR)
                nc.scalar.activation(out=qPack[0:E, he, 0, :], in_=psq[0:E, :],
                                     func=AF.Identity, bias=cq_pm[0:E, he:he + 1],
                                     scale=1.0)
                nc.scalar.activation(out=qPack[E:P, he, 1, :], in_=psq[E:P, :],
                                     func=AF.Identity, bias=cq_pm[E:P, he:he + 1],
                                     scale=1.0)

            for he in range(KO):
                emit_q(he)

            # ---------------- K/V per 1024-token half ----------------
            # (a single matmul output cannot cross a PSUM bank: N caps at 512)
            def emit_kv_half(half, ln_tiles):
                for rt in ln_tiles:
                    emit_ln_dma(rt)
                for he in range(KO):
                    for seg in range(2):
                        s_lo = half * (T // 2) + seg * 512
                        psk = ps_w.tile([P, 512], F32, tag="ps_w", name="psk")
                        for jj in range(KO // 2):
                            nc.tensor.matmul(
                                psk[:], wkAll[:, 2 * jj:2 * jj + 2, he * P:(he + 1) * P],
                                hT_full[:, 2 * jj:2 * jj + 2, s_lo:s_lo + 512],
                                start=(jj == 0), stop=(jj == KO // 2 - 1), perf_mode=DR)
                        nc.scalar.activation(out=kT[:, he, s_lo:s_lo + 512], in_=psk[:],
                                             func=AF.Identity, bias=ck_pm[:, he:he + 1],
                                             scale=1.0)
                for s in range(half * 8, half * 8 + 8):
                    for nh in range(2):
                        psv = ps_w.tile([P, 512], F32, tag="ps_w", name="psv")
                        for jj in range(KO // 2):
                            nc.tensor.matmul(
                                psv[:], hT_full[:, 2 * jj:2 * jj + 2, s * P:(s + 1) * P],
                                wvAll[:, 2 * jj:2 * jj + 2, nh * 512:(nh + 1) * 512],
                                start=(jj == 0), stop=(jj == KO // 2 - 1), perf_mode=DR)
                        nc.vector.tensor_copy(
                            out=vP[:, s // 2, s % 2, nh * 8:(nh + 1) * 8, 0:E],
                            in_=psv[:].rearrange("p (h e) -> p h e", e=E))

            emit_kv_half(0, [4, 5, 6, 7])
            emit_kv_half(1, [8, 9, 10, 11, 12, 13, 14, 15])

            # post-attention weights: DMAs enqueue now, land during attention
            nc.sync.dma_start(woAll[:], wo.rearrange("(o p) n -> p o n", p=P))
            nc.sync.dma_start(w1All[:], w1.rearrange("(o p) n -> p o n", p=P))
            nc.sync.dma_start(w2All[:], w2.rearrange("(o p) n -> p o n", p=P))

            xrow_ctx.__exit__(None, None, None)

            # ---------------- attention ----------------
            ps_w_ctx.__exit__(None, None, None)
            ps_tr_ctx.__exit__(None, None, None)
            exps_ctx = tc.tile_pool(name="exps", bufs=8)
            exps_pool = exps_ctx.__enter__()
            ps_qk_ctx = tc.tile_pool(name="ps_qk", bufs=3, space="PSUM")
            ps_qk = ps_qk_ctx.__enter__()
            ps_u_ctx = tc.tile_pool(name="ps_u", bufs=2, space="PSUM")
            ps_u = ps_u_ctx.__enter__()
            den_ctx = tc.tile_pool(name="denpool", bufs=2)
            den_pool = den_ctx.__enter__()

            def emit_attn(h, denG):
                pbase = (h % 2) * E
                ko_h = h // 2
                psu = ps_u.tile([P, 512], F32, tag="ps_u", name="psu")
                for sp in range(SO // 2):
                    pss = ps_qk.tile([P, 2, 512], F32, tag="ps_qk", name="pss")
                    for j in range(2):
                        so = 2 * sp + j
                        # full-K stationary (FWL-eligible); the other head's
                        # rows meet zeros in the packed q, so the sum is exact
                        nc.tensor.matmul(
                            pss[:, j, :],
                            kT[:, ko_h, so * P:(so + 1) * P],
                            qPack[:, ko_h, h % 2, :],
                            start=True, stop=True)
                    es = exps_pool.tile([P, 2, 512], FP8, tag="exps", name="es")
                    nc.scalar.activation(out=es[:], in_=pss[:], func=AF.Exp, scale=SCALE)
                    # fp8 DoubleRow: virtual K=256 sums both key chunks at once
                    nc.tensor.matmul(
                        psu[0:E + 1, :], vP[:, sp, :, h, :], es[:],
                        start=(sp == 0), stop=(sp == SO // 2 - 1),
                        perf_mode=DR)
                # Cheap PSUM evacuation: unnormalized U (bf16) + the
                # denominator row parked at partition 32*(h%4) of the group
                # staging tile (one reciprocal then serves 4 heads).
                j4 = 32 * (h % 4)
                nc.vector.tensor_copy(out=denG[j4:j4 + 1, :], in_=psu[E:E + 1, :])
                nc.vector.tensor_copy(out=oT[pbase:pbase + E, ko_h, :], in_=psu[0:E, :])

            for h in range(H):
                if h % 4 == 0:
                    denG = den_pool.tile([P, TQ], F32, tag="denG", name="denG")
                    # unused partitions must stay finite: recip(1)=1, and the
                    # selector matmul would turn 0*inf into NaN otherwise
                    nc.vector.memset(denG[:], 1.0)
                emit_attn(h, denG)
                if h % 4 == 3:
                    g = h - 3
                    dinvf = den_pool.tile([P, TQ], F32, tag="dinvf", name="dinvf")
                    nc.vector.reciprocal(out=dinvf[:], in_=denG[:])
                    dinvb = den_pool.tile([P, TQ], BF16, tag="dinvb", name="dinvb")
                    nc.vector.tensor_copy(out=dinvb[:], in_=dinvf[:])
                    for kk, sel in ((0, selK0), (1, selK1)):
                        ko_h = g // 2 + kk
                        psb = ps_qk.tile([P, 2, 512], F32, tag="ps_qk", name="psb")
                        nc.tensor.matmul(psb[:, 0, :], sel[:], dinvb[:],
                                         start=True, stop=True)
                        nc.vector.tensor_tensor(
                            out=oT[:, ko_h, :], in0=oT[:, ko_h, :],
                            in1=psb[:, 0, :], op=ALU.mult)

            den_ctx.__exit__(None, None, None)
            ps_u_ctx.__exit__(None, None, None)
            ps_qk_ctx.__exit__(None, None, None)
            exps_ctx.__exit__(None, None, None)
            evac_ctx = tc.tile_pool(name="evac2", bufs=3)
            evac_pool = evac_ctx.__enter__()
            ps_w_ctx = tc.tile_pool(name="ps_w2", bufs=2, space="PSUM")
            ps_w = ps_w_ctx.__enter__()
            ps_tr_ctx = tc.tile_pool(name="ps_tr2", bufs=2, space="PSUM")
            ps_tr = ps_tr_ctx.__enter__()

            # ---------------- Wo projection + residual + LN2 ----------------
            for mm in range(KO):
                psy = ps_w.tile([P, 512], F32, tag="ps_w")
                for ko in range(KO):
                    nc.tensor.matmul(
                        psy[:], woAll[:, ko, mm * P:(mm + 1) * P], oT[:, ko, :],
                        start=(ko == 0), stop=(ko == KO - 1))
                ysb = evac_pool.tile([P, 512], F32, tag="ysb")
                nc.vector.tensor_scalar_add(out=ysb[:], in0=psy[:], scalar1=bo_pm[:, mm:mm + 1])
                trp = ps_tr.tile([P, RT_OWN, P], F32, tag="tr")
                for rt in range(RT_OWN):
                    nc.tensor.transpose(trp[:, rt, :], ysb[:, rt * P:(rt + 1) * P], id_f32[:])
                nc.vector.tensor_tensor(
                    out=x1[:, :, mm * P:(mm + 1) * P],
                    in0=x1[:, :, mm * P:(mm + 1) * P], in1=trp[:], op=ALU.add)

            for rt in range(RT_OWN):
                y_row = hrow_pool.tile([P, D], BF16, tag="hrow")
                _layer_norm_rows(nc, sng, x1[:, rt, :], stats_pool, y_row[:])
                trp = ps_tr.tile([P, KO, P], BF16, tag="tr2")
                for ko in range(KO):
                    nc.tensor.transpose(trp[:, ko, :], y_row[:, ko * P:(ko + 1) * P], id_bf[:])
                nc.scalar.copy(out=h2T[:, :, rt * P:(rt + 1) * P], in_=trp[:])

            # ---------------- FFN ----------------
            for mm in range(KO):
                psf = ps_w.tile([P, 512], F32, tag="ps_w")
                for ko in range(KO):
                    nc.tensor.matmul(
                        psf[:], w1All[:, ko, mm * P:(mm + 1) * P], h2T[:, ko, :],
                        start=(ko == 0), stop=(ko == KO - 1))
                # f = gelu(x + b1), fused bias via activation
                nc.scalar.activation(out=fT[:, mm, :], in_=psf[:], func=AF.Gelu,
                                     bias=bf1_pm[:, mm:mm + 1], scale=1.0)
            for mm in range(KO):
                psz = ps_w.tile([P, 512], F32, tag="ps_w")
                for ko in range(KO):
                    nc.tensor.matmul(
                        psz[:], w2All[:, ko, mm * P:(mm + 1) * P], fT[:, ko, :],
                        start=(ko == 0), stop=(ko == KO - 1))
                zsb = evac_pool.tile([P, 512], F32, tag="ysb")
                nc.vector.tensor_scalar_add(out=zsb[:], in0=psz[:], scalar1=bf2_pm[:, mm:mm + 1])
                trp = ps_tr.tile([P, RT_OWN, P], F32, tag="tr")
                for rt in range(RT_OWN):
                    nc.tensor.transpose(trp[:, rt, :], zsb[:, rt * P:(rt + 1) * P], id_f32[:])
                nc.vector.tensor_tensor(
                    out=x1[:, :, mm * P:(mm + 1) * P],
                    in0=x1[:, :, mm * P:(mm + 1) * P], in1=trp[:], op=ALU.add)
                # stream the finished 128-column strip out now
                nc.sync.dma_start(
                    out.rearrange("(r p) d -> p r d", p=P)[:, :, mm * P:(mm + 1) * P],
                    x1[:, :, mm * P:(mm + 1) * P])

            ps_tr_ctx.__exit__(None, None, None)
            evac_ctx.__exit__(None, None, None)
            ps_w_ctx.__exit__(None, None, None)

    nc.compile()
    return nc


_NC_CACHE = None


def _get_nc():
    global _NC_CACHE
    if _NC_CACHE is None:
        _NC_CACHE = build_kernel()
    return _NC_CACHE


def _prep_weights(Wq, Wk, Wv, Wo, W1, W2, ln1_g, ln1_b, ln2_g, ln2_b, b1):
    """Fold LayerNorm gamma into the consuming weights and beta into bias
    vectors (exact math, done in f32 before the low-precision cast)."""
    bf = ml_dtypes.bfloat16
    f8 = ml_dtypes.float8_e4m3
    # [H, D, E] -> [D, H*E]
    wq = np.ascontiguousarray(np.transpose(Wq, (1, 0, 2)).reshape(D, D))
    wk = np.ascontiguousarray(np.transpose(Wk, (1, 0, 2)).reshape(D, D))
    wv = np.ascontiguousarray(np.transpose(Wv, (1, 0, 2)).reshape(D, D))
    cq = ln1_b @ wq
    ck = ln1_b @ wk
    cv = ln1_b @ wv              # v bias; o = softmax(..)@v + cv, folded into bo
    bo_adj = cv @ Wo             # caller adds this to bo
    b1_adj = b1 + ln2_b @ W1
    return ((wq * ln1_g[:, None]).astype(f8), (wk * ln1_g[:, None]).astype(f8),
            (wv * ln1_g[:, None]).astype(f8), Wo.astype(bf),
            (W1 * ln2_g[:, None]).astype(bf), W2.astype(bf),
            cq.astype(np.float32), ck.astype(np.float32),
            bo_adj.astype(np.float32), b1_adj.astype(np.float32))


def kernel(x, Wq, Wk, Wv, Wo, bo, ln1_g, ln1_b, ln2_g, ln2_b, W1, b1, W2, b2,
           _trace=False):
    x = np.asarray(x, dtype=np.float32)
    wq, wk, wv, wo, w1, w2, cq_v, ck_v, bo_extra, b1_adj = _prep_weights(
        np.asarray(Wq, np.float32), np.asarray(Wk, np.float32),
        np.asarray(Wv, np.float32), np.asarray(Wo, np.float32),
        np.asarray(W1, np.float32), np.asarray(W2, np.float32),
        np.asarray(ln1_g, np.float32), np.asarray(ln1_b, np.float32),
        np.asarray(ln2_g, np.float32), np.asarray(ln2_b, np.float32),
        np.asarray(b1, np.float32))
    common = {
        "wq": wq, "wk": wk, "wv": wv, "wo": wo, "w1": w1, "w2": w2,
        "cq": cq_v, "ck": ck_v,
        "bo": np.asarray(bo, np.float32) + bo_extra, "b1": b1_adj,
        "b2": np.asarray(b2, np.float32),
    }
    in_maps = []
    for core in range(8):
        b, c = divmod(core, 4)
        # rotate so this core's own 512 rows come first (keys are
        # permutation-invariant under softmax; queries are the own rows)
        xperm = np.ascontiguousarray(np.roll(x[b], -c * TQ, axis=0))
        in_maps.append({"xfull": xperm, **common})

    nc = _get_nc()
    res = run_bass_kernel_spmd(nc, in_maps, core_ids=list(range(8)), trace=_trace)
    out = np.empty((2, T, D), np.float32)
    for core in range(8):
        b, c = divmod(core, 4)
        out[b, c * TQ:(c + 1) * TQ] = res.results[core]["out"]
    if _trace:
        kernel.last_results = res
    return out


# revision 11
# speedup vs baseline: 1.1069x; 1.1069x over previous
"""Trainium2 Bass kernel for nn_Block_49624052138029 (dense transformer block).

Strategy: data parallel over batch x sequence: core i = (batch i//4, query
quarter i%4 of 512 tokens). K/V are computed redundantly on all 4 cores of a
batch group (measured faster than collectives: the gather would sit on the
critical path with nothing to overlap it).

Key-permutation trick: attention is permutation-invariant over keys, so each
core receives xfull ROTATED so its own 512 rows come first. One identical
program then serves all 8 cores: LN/K/V production starts on the rows the
core already owns, Q + the first attention chunks never wait on remote rows.

v2 schedule (this session):
- LN1 runs once over all 2048 rows, own rows first (the old kernel LN'd own
  rows twice); Q is emitted right after the own 4 row-tiles.
- A short warmup stream of junk matmuls pushes HAM to K=8/8 before the first
  transpose (transposes don't count as PE-busy for the clock gate).
- K/V projections run at N=1024 (fp8 DoubleRow, moving [128,2,1024]): half
  the matmul count of N=512, so LDWEIGHTS hides under the matmuls.
- The softmax denominator reciprocal runs batched over groups of 4 heads
  ([4,512] costs the same as [1,512]: DVE reciprocal is ~8 cyc/elem and
  lane-parallel) - the per-head version burned 53us of DVE.
- wo/w1/w2 live in dedicated SBUF tiles whose DMAs are issued before
  attention (the old rotating 2-buf pool serialized the w1/w2 loads behind
  the Wo/W1 consumers: 5us PE gaps + HAM cold windows).
- The final output DMA streams per 128-column strip as the FFN finishes it
  instead of waiting for the whole residual (16us -> ~3us tail).

Precision (validated vs the reference): QKV projections fp8 DoubleRow
(h, Wq/Wk/Wv in fp8e4m3); K^T and packed Q stored fp8; QK on fp8 operands;
exp() evacuation on ScalarE with fp8 out; AV in fp8 DoubleRow with an
all-ones V column so the softmax denominator falls out of the same
accumulation (row 64 of U^T). Post-attention dense layers (Wo/W1/W2) bf16.
"""

import os
import sys

for _p in ("/root/.axon_site", "/root/.axon_site/_ro/trn_rl_repo",
           "/root/.axon_site/_ro/pypackages", "/opt/trn_rl_repo", "/opt/pypackages"):
    if os.path.isdir(_p) and _p not in sys.path:
        sys.path.append(_p)

import numpy as np
import ml_dtypes

import concourse.bass as bass
import concourse.tile as tile
from concourse import bacc, mybir
from concourse.bass_utils import run_bass_kernel_spmd
from concourse.masks import make_identity

F32 = mybir.dt.float32
BF16 = mybir.dt.bfloat16
FP8 = mybir.dt.float8e4
AF = mybir.ActivationFunctionType
ALU = mybir.AluOpType
DR = mybir.MatmulPerfMode.DoubleRow

D = 1024          # model dim
H = 16            # heads
E = 64            # head dim
T = 2048          # tokens per batch
TQ = 512          # tokens owned by this core
P = 128
KO = D // P       # 8 feature chunks
RT_OWN = TQ // P  # 4 row tiles owned
SO = T // P       # 16 key chunks
EPS = 1e-5
SCALE = 1.0 / 32.0  # D ** -0.5

# K/V projection moving-operand width (fp8 DR). 1024 halves the matmul
# count vs 512; flip to 512 if the wide moving operand is rejected.
KV_N = 1024


def _layer_norm_rows(nc, sng, xrow_ap, stats_pool, y_out_ap):
    """Row-major LayerNorm core: y = (x - mean(x)) * rsqrt(var(x) + eps).

    xrow_ap: [128, 1024] f32 SBUF; y_out_ap: [128, 1024] (any dtype) SBUF.
    gamma/beta are NOT applied here (folded into the weights host-side).
    """
    stats = stats_pool.tile([P, 2, 6], F32, tag="bnstats")
    xg = xrow_ap.rearrange("p (g d) -> p g d", g=2)
    for g in range(2):
        nc.vector.bn_stats(out=stats[:, g, :], in_=xg[:, g, :])
    mv = stats_pool.tile([P, 2], F32, tag="bnaggr")
    nc.vector.bn_aggr(out=mv[:], in_=stats[:])
    rstd = stats_pool.tile([P, 1], F32, tag="rstd")
    # rstd = 1 / sqrt(var + eps)
    nc.scalar.activation(out=rstd[:], in_=mv[:, 1:2], func=AF.Sqrt,
                         bias=sng["eps"][:], scale=1.0)
    nc.vector.reciprocal(out=rstd[:], in_=rstd[:])
    nc.vector.tensor_scalar(
        out=y_out_ap, in0=xrow_ap, scalar1=mv[:, 0:1], scalar2=rstd[:],
        op0=ALU.subtract, op1=ALU.mult)


def build_kernel():
    nc = bacc.Bacc(None, target_bir_lowering=False, debug=False, num_devices=8)

    # xfull is PERMUTED per core: own 512 rows first, then the rest.
    xfull = nc.dram_tensor("xfull", [T, D], F32, kind="ExternalInput")
    wq = nc.dram_tensor("wq", [D, D], FP8, kind="ExternalInput")
    wk = nc.dram_tensor("wk", [D, D], FP8, kind="ExternalInput")
    wv = nc.dram_tensor("wv", [D, D], FP8, kind="ExternalInput")
    wo = nc.dram_tensor("wo", [D, D], BF16, kind="ExternalInput")
    w1 = nc.dram_tensor("w1", [D, D], BF16, kind="ExternalInput")
    w2 = nc.dram_tensor("w2", [D, D], BF16, kind="ExternalInput")
    cq = nc.dram_tensor("cq", [D], F32, kind="ExternalInput")
    ck = nc.dram_tensor("ck", [D], F32, kind="ExternalInput")
    bo = nc.dram_tensor("bo", [D], F32, kind="ExternalInput")
    b1 = nc.dram_tensor("b1", [D], F32, kind="ExternalInput")
    b2 = nc.dram_tensor("b2", [D], F32, kind="ExternalInput")
    out = nc.dram_tensor("out", [TQ, D], F32, kind="ExternalOutput")

    # per-feature params as [128, 8] (partition p, chunk o) for feature-major use
    def pm(dram_vec):
        return dram_vec.rearrange("(o p) -> p o", p=P)

    with tile.TileContext(nc) as tc:
        with (
            tc.tile_pool(name="singles", bufs=1) as singles,
            tc.tile_pool(name="persist", bufs=1) as persist,
            tc.tile_pool(name="hrow", bufs=4) as hrow_pool,
            tc.tile_pool(name="stats", bufs=6) as stats_pool,
        ):
            # ---------------- persistent activations + weights ----------------
            kT = persist.tile([P, KO, T], FP8, name="kT")           # 2 MB
            vP = persist.tile([P, SO // 2, 2, H, E + 1], FP8, name="vP")  # 2.08 MB
            qPack = persist.tile([P, KO, 2, TQ], FP8, name="qPack")  # 1 MB
            oT = persist.tile([P, KO, TQ], BF16, name="oT")         # 1 MB
            x1 = persist.tile([P, RT_OWN, D], F32, name="x1")       # 2 MB
            h2T = persist.tile([P, KO, TQ], BF16, name="h2T")       # 1 MB
            fT = persist.tile([P, KO, TQ], BF16, name="fT")         # 1 MB
            hT_full = persist.tile([P, KO, T], FP8, name="hT_full")  # 2 MB

            wqAll = persist.tile([P, KO, D], FP8, name="wqAll")     # 1 MB
            wkAll = persist.tile([P, KO, D], FP8, name="wkAll")     # 1 MB
            wvAll = persist.tile([P, KO, D], FP8, name="wvAll")     # 1 MB
            woAll = persist.tile([P, KO, D], BF16, name="woAll")    # 2 MB
            w1All = persist.tile([P, KO, D], BF16, name="w1All")    # 2 MB
            w2All = persist.tile([P, KO, D], BF16, name="w2All")    # 2 MB

            # x own rows first: LN1 heads every dependency chain
            for rt in range(RT_OWN):
                nc.sync.dma_start(x1[:, rt, :], xfull[rt * P:(rt + 1) * P, :])
            nc.sync.dma_start(wqAll[:], wq.rearrange("(o p) n -> p o n", p=P))
            nc.sync.dma_start(wkAll[:], wk.rearrange("(o p) n -> p o n", p=P))
            nc.sync.dma_start(wvAll[:], wv.rearrange("(o p) n -> p o n", p=P))

            # ---------------- HAM warmup ----------------
            # ~5us of junk matmuls (uninitialized operands, zero deps) so the
            # PE clock gate reaches K=8/8 before the first real matmul
            # (transposes do not count as PE-busy for HAM).
            junk = singles.tile([P, E], BF16, name="junk")
            nc.vector.memset(junk[:], 0.0)
            warm_ctx = tc.tile_pool(name="ps_warm", bufs=2, space="PSUM")
            ps_warm = warm_ctx.__enter__()
            for i in range(48):
                wt = ps_warm.tile([P, E], F32, tag="warm")
                nc.tensor.matmul(wt[0:E, :], junk[:], junk[:], start=True, stop=True)
            warm_ctx.__exit__(None, None, None)

            # ---------------- setup ----------------
            sng = {}
            id_bf = singles.tile([P, P], BF16, name="id_bf")
            make_identity(nc, id_bf[:])
            id_f32 = singles.tile([P, P], F32, name="id_f32")
            make_identity(nc, id_f32[:])
            sng["eps"] = singles.tile([P, 1], F32, name="eps")
            nc.vector.memset(sng["eps"][:], EPS)
            ones64 = singles.tile([1, E], BF16, name="ones64")
            nc.vector.memset(ones64[:], 1.0)

            cq_pm = singles.tile([P, KO], F32, name="cq_pm")
            nc.sync.dma_start(cq_pm[:], pm(cq))
            ck_pm = singles.tile([P, KO], F32, name="ck_pm")
            nc.sync.dma_start(ck_pm[:], pm(ck))
            bo_pm = singles.tile([P, KO], F32, name="bo_pm")
            nc.sync.dma_start(bo_pm[:], pm(bo))
            bf1_pm = singles.tile([P, KO], F32, name="bf1_pm")
            nc.sync.dma_start(bf1_pm[:], pm(b1))
            bf2_pm = singles.tile([P, KO], F32, name="bf2_pm")
            nc.sync.dma_start(bf2_pm[:], pm(b2))

            # head-group denominator staging: head h of a group of 4 parks its
            # denominator row at partition 32*(h%4) so one [128,TQ] reciprocal
            # covers 4 heads (DVE reciprocal cost scales with free size only).
            # selK0/selK1 broadcast the dinv rows back over the oT partitions:
            # out[p,q] = sum_k sel[k,p] * dinv[k,q], sel one-hot per 64-row half.
            selK0 = singles.tile([P, P], BF16, name="selK0")
            nc.vector.memset(selK0[:], 0.0)
            nc.vector.memset(selK0[0:1, 0:E], 1.0)
            nc.vector.memset(selK0[32:33, E:P], 1.0)
            selK1 = singles.tile([P, P], BF16, name="selK1")
            nc.vector.memset(selK1[:], 0.0)
            nc.vector.memset(selK1[64:65, 0:E], 1.0)
            nc.vector.memset(selK1[96:97, E:P], 1.0)

            ps_tr_ctx = tc.tile_pool(name="ps_tr", bufs=2, space="PSUM")
            ps_tr = ps_tr_ctx.__enter__()
            ps_w_ctx = tc.tile_pool(name="ps_w", bufs=2, space="PSUM")
            ps_w = ps_w_ctx.__enter__()
            xrow_ctx = tc.tile_pool(name="xrow", bufs=3)
            xrow_pool = xrow_ctx.__enter__()

            # ---------------- LN1 over all rows -> hT_full ----------------
            def ln_tile_from(x_ap, rt):
                """LayerNorm one 128-row tile (source already in SBUF) and
                scatter its transpose into hT_full columns rt*128..+128."""
                y_row = hrow_pool.tile([P, D], BF16, tag="hrow")
                _layer_norm_rows(nc, sng, x_ap, stats_pool, y_row[:])
                trp = ps_tr.tile([P, KO, P], BF16, tag="tr")
                for ko in range(KO):
                    nc.tensor.transpose(trp[:, ko, :], y_row[:, ko * P:(ko + 1) * P], id_bf[:])
                nc.scalar.copy(out=hT_full[:, :, rt * P:(rt + 1) * P], in_=trp[:])

            def emit_ln_dma(rt):
                x_t = xrow_pool.tile([P, D], F32, tag="xrow")
                nc.sync.dma_start(x_t[:], xfull[rt * P:(rt + 1) * P, :])
                ln_tile_from(x_t[:], rt)

            # own rows (already being DMA'd into x1)
            for rt in range(RT_OWN):
                ln_tile_from(x1[:, rt, :], rt)

            # memsets: after the LN emission so the DVE queue reaches the
            # first bn_stats immediately; before Q/attention which need them
            nc.vector.memset(qPack[:], 0.0)
            nc.vector.memset(vP[:, :, :, :, E], 1.0)

            # ---------------- Q over own rows ----------------
            def emit_q(he):
                psq = ps_w.tile([P, TQ], F32, tag="psq", name="psq")
                for jj in range(KO // 2):
                    nc.tensor.matmul(
                        psq[:], wqAll[:, 2 * jj:2 * jj + 2, he * P:(he + 1) * P],
                        hT_full[:, 2 * jj:2 * jj + 2, 0:TQ],
                        start=(jj == 0), stop=(jj == KO // 2 - 1), perf_mode=DR)
                nc.scalar.activation(out=qPack[0:E, he, 0, :], in_=psq[0:E, :],
                                     func=AF.Identity, bias=cq_pm[0:E, he:he + 1],
                                     scale=1.0)
                nc.scalar.activation(out=qPack[E:P, he, 1, :], in_=psq[E:P, :],
                                     func=AF.Identity, bias=cq_pm[E:P, he:he + 1],
                                     scale=1.0)

            for he in range(KO):
                emit_q(he)

            # ---------------- K/V per 512-token group ----------------
            # (a single matmul output cannot cross a PSUM bank: N caps at 512)
            def emit_kv_group(g):
                s_lo = g * 512
                for he in range(KO):
                    psk = ps_w.tile([P, 512], F32, tag="ps_w", name="psk")
                    for jj in range(KO // 2):
                        nc.tensor.matmul(
                            psk[:], wkAll[:, 2 * jj:2 * jj + 2, he * P:(he + 1) * P],
                            hT_full[:, 2 * jj:2 * jj + 2, s_lo:s_lo + 512],
                            start=(jj == 0), stop=(jj == KO // 2 - 1), perf_mode=DR)
                    nc.scalar.activation(out=kT[:, he, s_lo:s_lo + 512], in_=psk[:],
                                         func=AF.Identity, bias=ck_pm[:, he:he + 1],
                                         scale=1.0)
                for s in range(g * 4, g * 4 + 4):
                    for nh in range(2):
                        psv = ps_w.tile([P, 512], F32, tag="ps_w", name="psv")
                        for jj in range(KO // 2):
                            nc.tensor.matmul(
                                psv[:], hT_full[:, 2 * jj:2 * jj + 2, s * P:(s + 1) * P],
                                wvAll[:, 2 * jj:2 * jj + 2, nh * 512:(nh + 1) * 512],
                                start=(jj == 0), stop=(jj == KO // 2 - 1), perf_mode=DR)
                        nc.vector.tensor_copy(
                            out=vP[:, s // 2, s % 2, nh * 8:(nh + 1) * 8, 0:E],
                            in_=psv[:].rearrange("p (h e) -> p h e", e=E))

            emit_kv_group(0)
            for g in range(1, 4):
                for rt in range(4 * g, 4 * g + 4):
                    emit_ln_dma(rt)
                emit_kv_group(g)

            # post-attention weights: DMAs enqueue now, land during attention
            nc.sync.dma_start(woAll[:], wo.rearrange("(o p) n -> p o n", p=P))
            nc.sync.dma_start(w1All[:], w1.rearrange("(o p) n -> p o n", p=P))
            nc.sync.dma_start(w2All[:], w2.rearrange("(o p) n -> p o n", p=P))

            xrow_ctx.__exit__(None, None, None)

            # ---------------- attention ----------------
            ps_w_ctx.__exit__(None, None, None)
            ps_tr_ctx.__exit__(None, None, None)
            exps_ctx = tc.tile_pool(name="exps", bufs=8)
            exps_pool = exps_ctx.__enter__()
            ps_qk_ctx = tc.tile_pool(name="ps_qk", bufs=3, space="PSUM")
            ps_qk = ps_qk_ctx.__enter__()
            ps_u_ctx = tc.tile_pool(name="ps_u", bufs=2, space="PSUM")
            ps_u = ps_u_ctx.__enter__()
            den_ctx = tc.tile_pool(name="denpool", bufs=2)
            den_pool = den_ctx.__enter__()

            def emit_attn(h, denG):
                pbase = (h % 2) * E
                ko_h = h // 2
                psu = ps_u.tile([P, 512], F32, tag="ps_u", name="psu")
                for sp in range(SO // 2):
                    pss = ps_qk.tile([P, 2, 512], F32, tag="ps_qk", name="pss")
                    for j in range(2):
                        so = 2 * sp + j
                        # full-K stationary (FWL-eligible); the other head's
                        # rows meet zeros in the packed q, so the sum is exact
                        nc.tensor.matmul(
                            pss[:, j, :],
                            kT[:, ko_h, so * P:(so + 1) * P],
                            qPack[:, ko_h, h % 2, :],
                            start=True, stop=True)
                    es = exps_pool.tile([P, 2, 512], FP8, tag="exps", name="es")
                    nc.scalar.activation(out=es[:], in_=pss[:], func=AF.Exp, scale=SCALE)
                    # fp8 DoubleRow: virtual K=256 sums both key chunks at once
                    nc.tensor.matmul(
                        psu[0:E + 1, :], vP[:, sp, :, h, :], es[:],
                        start=(sp == 0), stop=(sp == SO // 2 - 1),
                        perf_mode=DR)
                # Cheap PSUM evacuation: unnormalized U (bf16) + the
                # denominator row parked at partition 32*(h%4) of the group
                # staging tile (one reciprocal then serves 4 heads).
                j4 = 32 * (h % 4)
                nc.vector.tensor_copy(out=denG[j4:j4 + 1, :], in_=psu[E:E + 1, :])
                nc.vector.tensor_copy(out=oT[pbase:pbase + E, ko_h, :], in_=psu[0:E, :])

            def emit_group_epi(g, denG):
                dinvf = den_pool.tile([P, TQ], F32, tag="dinvf", name="dinvf")
                nc.vector.reciprocal(out=dinvf[:], in_=denG[:])
                dinvb = den_pool.tile([P, TQ], BF16, tag="dinvb", name="dinvb")
                nc.vector.tensor_copy(out=dinvb[:], in_=dinvf[:])
                for kk, sel in ((0, selK0), (1, selK1)):
                    ko_h = 2 * g + kk
                    psb = ps_qk.tile([P, 2, 512], F32, tag="ps_qk", name="psb")
                    nc.tensor.matmul(psb[:, 0, :], sel[:], dinvb[:],
                                     start=True, stop=True)
                    nc.vector.tensor_tensor(
                        out=oT[:, ko_h, :], in0=oT[:, ko_h, :],
                        in1=psb[:, 0, :], op=ALU.mult)

            denGs = {}
            for h in range(H):
                if h % 4 == 0:
                    denGs[h // 4] = den_pool.tile([P, TQ], F32, tag="denG", name="denG")
                    # unused partitions must stay finite: recip(1)=1, and the
                    # selector matmul would turn 0*inf into NaN otherwise
                    nc.vector.memset(denGs[h // 4][:], 1.0)
                emit_attn(h, denGs[h // 4])
                # group g's epilogue lands 2 heads into the following group:
                # the reciprocal chain overlaps those heads' QK/exp instead of
                # stalling the PE at the group boundary
                if h >= 5 and (h - 5) % 4 == 0:
                    emit_group_epi((h - 5) // 4, denGs[(h - 5) // 4])
            emit_group_epi(3, denGs[3])

            den_ctx.__exit__(None, None, None)
            ps_u_ctx.__exit__(None, None, None)
            ps_qk_ctx.__exit__(None, None, None)
            exps_ctx.__exit__(None, None, None)
            evac_ctx = tc.tile_pool(name="evac2", bufs=3)
            evac_pool = evac_ctx.__enter__()
            ps_w_ctx = tc.tile_pool(name="ps_w2", bufs=2, space="PSUM")
            ps_w = ps_w_ctx.__enter__()
            ps_tr_ctx = tc.tile_pool(name="ps_tr2", bufs=2, space="PSUM")
            ps_tr = ps_tr_ctx.__enter__()

            # ---------------- Wo projection + residual + LN2 ----------------
            for mm in range(KO):
                psy = ps_w.tile([P, 512], F32, tag="ps_w")
                for ko in range(KO):
                    nc.tensor.matmul(
                        psy[:], woAll[:, ko, mm * P:(mm + 1) * P], oT[:, ko, :],
                        start=(ko == 0), stop=(ko == KO - 1))
                ysb = evac_pool.tile([P, 512], F32, tag="ysb")
                nc.vector.tensor_scalar_add(out=ysb[:], in0=psy[:], scalar1=bo_pm[:, mm:mm + 1])
                trp = ps_tr.tile([P, RT_OWN, P], F32, tag="tr")
                for rt in range(RT_OWN):
                    nc.tensor.transpose(trp[:, rt, :], ysb[:, rt * P:(rt + 1) * P], id_f32[:])
                nc.vector.tensor_tensor(
                    out=x1[:, :, mm * P:(mm + 1) * P],
                    in0=x1[:, :, mm * P:(mm + 1) * P], in1=trp[:], op=ALU.add)

            for rt in range(RT_OWN):
                y_row = hrow_pool.tile([P, D], BF16, tag="hrow")
                _layer_norm_rows(nc, sng, x1[:, rt, :], stats_pool, y_row[:])
                trp = ps_tr.tile([P, KO, P], BF16, tag="tr2")
                for ko in range(KO):
                    nc.tensor.transpose(trp[:, ko, :], y_row[:, ko * P:(ko + 1) * P], id_bf[:])
                nc.scalar.copy(out=h2T[:, :, rt * P:(rt + 1) * P], in_=trp[:])

            # ---------------- FFN ----------------
            for mm in range(KO):
                psf = ps_w.tile([P, 512], F32, tag="ps_w")
                for ko in range(KO):
                    nc.tensor.matmul(
                        psf[:], w1All[:, ko, mm * P:(mm + 1) * P], h2T[:, ko, :],
                        start=(ko == 0), stop=(ko == KO - 1))
                # f = gelu(x + b1), fused bias via activation
                nc.scalar.activation(out=fT[:, mm, :], in_=psf[:], func=AF.Gelu,
                                     bias=bf1_pm[:, mm:mm + 1], scale=1.0)
            for mm in range(KO):
                psz = ps_w.tile([P, 512], F32, tag="ps_w")
                for ko in range(KO):
                    nc.tensor.matmul(
                        psz[:], w2All[:, ko, mm * P:(mm + 1) * P], fT[:, ko, :],
                        start=(ko == 0), stop=(ko == KO - 1))
                zsb = evac_pool.tile([P, 512], F32, tag="ysb")
                nc.vector.tensor_scalar_add(out=zsb[:], in0=psz[:], scalar1=bf2_pm[:, mm:mm + 1])
                trp = ps_tr.tile([P, RT_OWN, P], F32, tag="tr")
                for rt in range(RT_OWN):
                    nc.tensor.transpose(trp[:, rt, :], zsb[:, rt * P:(rt + 1) * P], id_f32[:])
                nc.vector.tensor_tensor(
                    out=x1[:, :, mm * P:(mm + 1) * P],
                    in0=x1[:, :, mm * P:(mm + 1) * P], in1=trp[:], op=ALU.add)
                # stream the finished 128-column strip out now
                nc.sync.dma_start(
                    out.rearrange("(r p) d -> p r d", p=P)[:, :, mm * P:(mm + 1) * P],
                    x1[:, :, mm * P:(mm + 1) * P])

            ps_tr_ctx.__exit__(None, None, None)
            evac_ctx.__exit__(None, None, None)
            ps_w_ctx.__exit__(None, None, None)

    nc.compile()
    return nc


_NC_CACHE = None


def _get_nc():
    global _NC_CACHE
    if _NC_CACHE is None:
        _NC_CACHE = build_kernel()
    return _NC_CACHE


def _prep_weights(Wq, Wk, Wv, Wo, W1, W2, ln1_g, ln1_b, ln2_g, ln2_b, b1):
    """Fold LayerNorm gamma into the consuming weights and beta into bias
    vectors (exact math, done in f32 before the low-precision cast)."""
    bf = ml_dtypes.bfloat16
    f8 = ml_dtypes.float8_e4m3
    # [H, D, E] -> [D, H*E]
    wq = np.ascontiguousarray(np.transpose(Wq, (1, 0, 2)).reshape(D, D))
    wk = np.ascontiguousarray(np.transpose(Wk, (1, 0, 2)).reshape(D, D))
    wv = np.ascontiguousarray(np.transpose(Wv, (1, 0, 2)).reshape(D, D))
    cq = ln1_b @ wq
    ck = ln1_b @ wk
    cv = ln1_b @ wv              # v bias; o = softmax(..)@v + cv, folded into bo
    bo_adj = cv @ Wo             # caller adds this to bo
    b1_adj = b1 + ln2_b @ W1
    return ((wq * ln1_g[:, None]).astype(f8), (wk * ln1_g[:, None]).astype(f8),
            (wv * ln1_g[:, None]).astype(f8), Wo.astype(bf),
            (W1 * ln2_g[:, None]).astype(bf), W2.astype(bf),
            cq.astype(np.float32), ck.astype(np.float32),
            bo_adj.astype(np.float32), b1_adj.astype(np.float32))


def kernel(x, Wq, Wk, Wv, Wo, bo, ln1_g, ln1_b, ln2_g, ln2_b, W1, b1, W2, b2,
           _trace=False):
    x = np.asarray(x, dtype=np.float32)
    wq, wk, wv, wo, w1, w2, cq_v, ck_v, bo_extra, b1_adj = _prep_weights(
        np.asarray(Wq, np.float32), np.asarray(Wk, np.float32),
        np.asarray(Wv, np.float32), np.asarray(Wo, np.float32),
        np.asarray(W1, np.float32), np.asarray(W2, np.float32),
        np.asarray(ln1_g, np.float32), np.asarray(ln1_b, np.float32),
        np.asarray(ln2_g, np.float32), np.asarray(ln2_b, np.float32),
        np.asarray(b1, np.float32))
    common = {
        "wq": wq, "wk": wk, "wv": wv, "wo": wo, "w1": w1, "w2": w2,
        "cq": cq_v, "ck": ck_v,
        "bo": np.asarray(bo, np.float32) + bo_extra, "b1": b1_adj,
        "b2": np.asarray(b2, np.float32),
    }
    in_maps = []
    for core in range(8):
        b, c = divmod(core, 4)
        # rotate so this core's own 512 rows come first (keys are
        # permutation-invariant under softmax; queries are the own rows)
        xperm = np.ascontiguousarray(np.roll(x[b], -c * TQ, axis=0))
        in_maps.append({"xfull": xperm, **common})

    nc = _get_nc()
    res = run_bass_kernel_spmd(nc, in_maps, core_ids=list(range(8)), trace=_trace)
    out = np.empty((2, T, D), np.float32)
    for core in range(8):
        b, c = divmod(core, 4)
        out[b, c * TQ:(c + 1) * TQ] = res.results[core]["out"]
    if _trace:
        kernel.last_results = res
    return out
